# revision 1
# baseline (speedup 1.0000x reference)
"""Trainium2 Bass kernel for a gated bilinear-attention GNN (GAT-with-gate).

Math (per batch b):
    h   = x @ W_w.T + W_b                      [N, D]
    e   = (h A) h^T ; e_sym = e + e^T = h (A + A^T) h^T   (one quadratic form)
    att = softmax(where(adj>0, e_sym, 0), axis=1) * adj
    rv  = h; 3x: az = relu(att @ rv);  c = sigmoid([h, az] @ gate_w.T + gate_b)
               rv = c * h + (1 - c) * az

Data-parallel over the batch dim, 2 batches per core on 8 cores.  Layouts:
    attT[j, i] = adj[i, j] * exp(e_sym[j, i])      (bf16, unnormalized)
    denom_j    = masked-exp row sums + (N - indeg_j) metadata
    azT[f, i]  = sum_j (rv[j, f]/denom_j) * attT[j, i]    (1/denom folded
                 into the stationary operand via the rvs/w1/w2 scaling)
    rv_new     = w1*h + w2*az in natural layout,  w1 = c/denom, w2 = (1-c)/denom

Key implementation points (evolved against perfetto traces; 97us -> 88us):
  - PSUM as 2-bank [128,1024] tiles so every ACT drain (exp, relu, h, hST,
    hnat) is ONE instruction per [128,1024] - ACT ops cost ~350ns fixed.
  - The az->natural transposes are regular bf16 matmuls against [I | gw2]
    (129 cols): each transpose also emits that block's gate az-term as a
    129th PSUM column.  No [1,N] gate matmuls, no coefficient transposes,
    no [1,512] ACT exps.  gh (gate h-term) is 8 tiny matmuls per batch.
  - adj travels as uint8 (host pre-permuted so DMA runs are 8KB); the
    mask+denominator is one full-slab DVE scalar_tensor_tensor with
    accum_out.  att/rvs/azT in bf16 (rel err stays ~2.5e-4).
  - per-slab denominator -> inv -> rvs so hop0 starts without waiting for
    the whole attention phase; rvs/azs alternate between ACT and DVE.
  - the PE (HAM) clock ramps over ~50us regardless of load (DVFS steps
    every ~10.24us; ramp is NOT accelerated by busy work - warm-up
    matmuls measurably hurt).  So: no fillers, elementwise-heavy work
    front-loaded, matmul-heavy hops late.
  - few coarse DMAs (const blob, xT+ndeg blob, adj halves, out halves);
    SP DIRECT2D dispatch is ~600ns per dma_start.

_fixup_waits post-processes the scheduled program to satisfy this walrus
build's one-sync-wait-per-instruction limit.
"""

import sys
from contextlib import ExitStack

import numpy as np

sys.path.insert(0, "/opt/trn_rl_repo")

import concourse.bass as bass
import concourse.tile as tile
from concourse import mybir
from concourse.bass_utils import run_bass_kernel_spmd


B, N, D = 16, 1024, 128
NCORES = 8
BPC = B // NCORES        # batches per core
NB = N // 128            # 128-row blocks per matrix dim
F32 = mybir.dt.float32
F32R = mybir.dt.float32r
BF16 = mybir.dt.bfloat16
OP = mybir.AluOpType
AF = mybir.ActivationFunctionType

# const blob column layout
C_ID, C_WW, C_WB, C_A, C_GW, C_NGB, C_V = 0, 128, 256, 257, 385, 387, 388
C_COLS = 389


def build_nc():
    nc = bass.Bass("TRN2", target_bir_lowering=False, debug=False,
                   num_devices=NCORES)

    cblob = nc.dram_tensor("cblob", [128, C_COLS], F32, kind="ExternalInput").ap()
    xTn = nc.dram_tensor("xTn", [BPC, D, N + 2 * NB], F32,
                         kind="ExternalInput").ap()
    adjP = nc.dram_tensor("adjP", [BPC, 128, NB * N], mybir.dt.uint8,
                          kind="ExternalInput").ap()
    out = nc.dram_tensor("out", [BPC, 128, N], F32, kind="ExternalOutput").ap()

    with tile.TileContext(nc) as tc, ExitStack() as ctx:
        consts = ctx.enter_context(tc.tile_pool(name="consts", bufs=1))
        ps_e = ctx.enter_context(tc.tile_pool(name="ps_e", bufs=2, space="PSUM"))
        ps_g2 = ctx.enter_context(tc.tile_pool(name="ps_g2", bufs=3, space="PSUM"))
        ps_ct = ctx.enter_context(tc.tile_pool(name="ps_ct", bufs=1, space="PSUM"))
        adj_pool = ctx.enter_context(tc.tile_pool(name="adj", bufs=2))
        att_pool = ctx.enter_context(tc.tile_pool(name="att", bufs=2))
        work = ctx.enter_context(tc.tile_pool(name="work", bufs=2))
        hop = ctx.enter_context(tc.tile_pool(name="hop", bufs=4))

        # ---- constants: one DMA, then on-chip prep ----------------------
        cb = consts.tile([128, C_COLS], F32, tag="cb")
        nc.sync.dma_start(cb[:, :], cblob[:, :])
        ident = cb[:, C_ID:C_ID + 128]
        wb_sb = cb[:, C_WB:C_WB + 1]
        v_sb = cb[:, C_V:C_V + 1]
        ngb_sb = cb[:, C_NGB:C_NGB + 1]

        identr = consts.tile([128, 128], F32R, tag="identr")
        nc.vector.tensor_copy(identr[:, :], ident)
        wwT_sb = consts.tile([D, D], F32R, tag="wwT")
        nc.vector.tensor_copy(wwT_sb[:, :], cb[:, C_WW:C_WW + 128])
        gwr_sb = consts.tile([D, 2], F32R, tag="gwr")
        nc.vector.tensor_copy(gwr_sb[:, :], cb[:, C_GW:C_GW + 2])
        # [I | gw2] in bf16: transpose+gate fused matmul moving operand
        identg = consts.tile([128, 129], BF16, tag="identg")
        nc.vector.tensor_copy(identg[:, 0:128], ident)
        nc.vector.tensor_copy(identg[:, 128:129], cb[:, C_GW + 1:C_GW + 2])

        m_sb = consts.tile([D, D], F32R, tag="mmat")
        nc.vector.tensor_copy(m_sb[:, :], cb[:, C_A:C_A + 128])

        # ---- input DMAs (few, coarse, contiguous) ------------------------
        xTn_sb = [None] * BPC
        adj_sb = [None] * BPC
        for b in range(BPC):
            xTn_sb[b] = work.tile([D, N + 2 * NB], F32R, tag="xTn",
                                  name="xTn_sb")
            nc.gpsimd.dma_start(xTn_sb[b][:, :], xTn[b, :, :])
            adj_sb[b] = adj_pool.tile([128, NB * N], mybir.dt.uint8,
                                      tag="adj", name="adj_sb")
            for hh in range(2):
                sl = slice(hh * 4 * N, (hh + 1) * 4 * N)
                nc.sync.dma_start(adj_sb[b][:, sl], adjP[b, :, sl])


        def phase_prologue(b, st):
            xT = xTn_sb[b]
            # pT[d', n] = sum_d M[d, d'] xT[d, n] + v[d']   (M = W^T S W,
            # symmetric, host-precomputed): e[j,i] = pT[:,j].xT[:,i] + q_j,
            # so the attention scores never wait on the h chain.
            pT_sb = work.tile([D, N], F32R, tag="pT")
            ph = ps_e.tile([128, N], F32, tag="ps_e")
            for ih in range(2):
                nc.tensor.matmul(ph[:, ih * 512:(ih + 1) * 512], m_sb[:, :],
                                 xT[:, ih * 512:(ih + 1) * 512],
                                 start=True, stop=True)
            nc.scalar.activation(pT_sb[:, :], ph[:, :], AF.Identity,
                                 bias=v_sb, scale=1.0)

            # hT[o, n] = sum_d WwT[d, o] xT[d, n] + Wb[o]
            hT_sb = work.tile([D, N], F32R, tag="hT")
            ph = ps_e.tile([128, N], F32, tag="ps_e")
            for ih in range(2):
                nc.tensor.matmul(ph[:, ih * 512:(ih + 1) * 512], wwT_sb[:, :],
                                 xT[:, ih * 512:(ih + 1) * 512],
                                 start=True, stop=True)
            nc.scalar.activation(hT_sb[:, :], ph[:, :],
                                 AF.Identity, bias=wb_sb, scale=1.0)

            # h in natural layout [node-in-block, nb*128 + f]
            hnat_sb = work.tile([128, N], F32R, tag="hnat")
            pt = ps_e.tile([128, N], F32R, tag="ps_e")
            for nb in range(NB):
                nc.tensor.transpose(pt[:, nb * 128:(nb + 1) * 128],
                                    hT_sb[:, nb * 128:(nb + 1) * 128],
                                    identr[:, :])
            nc.scalar.copy(hnat_sb[:, :], pt[:, :])

            # gh[node, nb] = sum_o gw1[o] hT[o, node]  (gate h-term).
            # 2-col moving operand: 1-col f32r moving fails the ISA check.
            gh_ps = ps_ct.tile([128, 2 * NB], F32, tag="ps_ct")
            for nb in range(NB):
                nc.tensor.matmul(gh_ps[:, 2 * nb:2 * nb + 2],
                                 hT_sb[:, nb * 128:(nb + 1) * 128],
                                 gwr_sb[:, 0:2], start=True, stop=True)
            gh_sb = work.tile([128, NB], F32, tag="gh")
            nc.vector.tensor_copy(gh_sb[:, :], gh_ps[:, 0:2 * NB:2])

            st.update(hT=hT_sb, pT=pT_sb, hnat=hnat_sb, gh=gh_sb,
                      ndeg=xT[:, N:N + NB].bitcast(F32),
                      qT=xT[:, N + NB:N + 2 * NB].bitcast(F32), xT=xT)

        def phase_att(b, st):
            # attT[j, i] = adj[i, j] * exp(e_sym[j, i])  (bf16, unnormalized)
            pT_sb, xT = st["pT"], st["xT"]
            qT = st["qT"]
            adjb = adj_sb[b]
            attT_sb = att_pool.tile([128, NB * N], BF16, tag="att")
            acc_sb = work.tile([D, NB], F32, tag="acc")
            inv_sb = work.tile([D, NB], F32, tag="inv")
            rvs = hop.tile([128, N], BF16, tag="rvs")
            hnat_sb = st["hnat"]
            for jb in range(NB):
                texp = work.tile([128, N], BF16, tag="texp", bufs=3)
                pe = ps_e.tile([128, N], F32, tag="ps_e")
                for ih in range(2):
                    nc.tensor.matmul(pe[:, ih * 512:(ih + 1) * 512],
                                     pT_sb[:, jb * 128:(jb + 1) * 128],
                                     xT[:, ih * 512:(ih + 1) * 512],
                                     start=True, stop=True)
                nc.scalar.activation(texp[:, :], pe[:, :], AF.Exp,
                                     bias=qT[:, jb:jb + 1], scale=1.0)
                nc.vector.scalar_tensor_tensor(
                    attT_sb[:, jb * N:(jb + 1) * N], texp[:, :], 1.0,
                    adjb[:, jb * N:(jb + 1) * N], OP.mult, OP.mult,
                    accum_out=acc_sb[:, jb:jb + 1])
                # per-slab denom -> inv -> rvs block: keeps the att->hop0
                # dependency chain short (no wait for all 8 accumulations)
                nc.vector.tensor_scalar(
                    inv_sb[:, jb:jb + 1], acc_sb[:, jb:jb + 1],
                    st["ndeg"][:, jb:jb + 1], None, OP.add)
                nc.vector.reciprocal(inv_sb[:, jb:jb + 1],
                                     inv_sb[:, jb:jb + 1])
                if jb % 2 == 0:
                    nc.vector.tensor_scalar_mul(
                        rvs[:, jb * 128:(jb + 1) * 128],
                        hnat_sb[:, jb * 128:(jb + 1) * 128],
                        inv_sb[:, jb:jb + 1])
                else:
                    nc.scalar.activation(rvs[:, jb * 128:(jb + 1) * 128],
                                         hnat_sb[:, jb * 128:(jb + 1) * 128],
                                         AF.Copy, scale=inv_sb[:, jb:jb + 1])
            st.update(att=attT_sb, inv=inv_sb, rv=rvs)

        # fused-transpose tile layout: 3 psum tiles of (3, 3, 2) blocks,
        # each block 129 cols (128 transpose + 1 gate column)
        PT_BLKS = (3, 3, 2)

        def phase_hop(b, st, k):
            last = (k == 2)
            hnat_sb, gh_sb = st["hnat"], st["gh"]
            attT_sb, rv = st["att"], st["rv"]
            # azT[f, i] = sum_j rvs[j, f] attT[j, i]
            azT_sb = hop.tile([128, N], BF16, tag="azT", bufs=2)
            paz = ps_e.tile([128, N], F32, tag="ps_e")
            for ih in range(2):
                for jb in range(NB):
                    nc.tensor.matmul(
                        paz[:, ih * 512:(ih + 1) * 512],
                        rv[:, jb * 128:(jb + 1) * 128],
                        attT_sb[:, jb * N + ih * 512: jb * N + (ih + 1) * 512],
                        start=(jb == 0), stop=(jb == NB - 1))
            nc.scalar.activation(azT_sb[:, :], paz[:, :], AF.Relu)

            # az to natural layout via bf16 matmul against [I | gw2]: each
            # 129-col block = transposed az + that block's gate az-term.
            pts = []
            en_in = hop.tile([128, NB], F32, tag="en_in", bufs=2)
            nb0 = 0
            for nblk in PT_BLKS:
                pt = ps_g2.tile([128, 512], F32, tag="ps_g2", name="pt")
                for t in range(nblk):
                    nb = nb0 + t
                    nc.tensor.matmul(pt[:, t * 129:(t + 1) * 129],
                                     azT_sb[:, nb * 128:(nb + 1) * 128],
                                     identg[:, :], start=True, stop=True)
                # gate columns + gh -> sigmoid input (positive sense)
                nc.vector.tensor_tensor(
                    en_in[:, nb0:nb0 + nblk],
                    pt[:, 128:129 * nblk:129],
                    gh_sb[:, nb0:nb0 + nblk], OP.add)
                pts.append((pt, nb0, nblk))
                nb0 += nblk

            # coeff c = sigmoid(en_in + gb) computed as 1/(1+exp(-x));
            # w1 = c (*1/denom unless last), w2 = 1-c = e*c (*...)
            en_sb = hop.tile([128, NB], F32, tag="en", bufs=2)
            nc.scalar.activation(en_sb[:, :], en_in[:, :], AF.Exp,
                                 bias=ngb_sb, scale=-1.0)
            w1 = hop.tile([128, NB], F32, tag="w1", bufs=2)
            w2 = hop.tile([128, NB], F32, tag="w2", bufs=2)
            nc.vector.tensor_scalar(w1[:, :], en_sb[:, :], 1.0, None, OP.add)
            nc.vector.reciprocal(w1[:, :], w1[:, :])
            nc.vector.tensor_tensor(w2[:, :], en_sb[:, :], w1[:, :], OP.mult)
            if not last:
                nc.vector.tensor_tensor(w1[:, :], w1[:, :], st["inv"], OP.mult)
                nc.vector.tensor_tensor(w2[:, :], w2[:, :], st["inv"], OP.mult)

            # combine: rv_new = w1*h + w2*az  (natural layout, per block)
            rv_new = hop.tile([128, N], F32 if last else BF16, tag="rvs")
            azs = hop.tile([128, N], BF16, tag="azs", bufs=2)
            for pt, nb0, nblk in pts:
                for t in range(nblk):
                    nb = nb0 + t
                    sl = slice(nb * 128, (nb + 1) * 128)
                    if nb % 2 == 0:
                        nc.vector.tensor_scalar_mul(
                            azs[:, sl], pt[:, t * 129:t * 129 + 128],
                            w2[:, nb:nb + 1])
                    else:
                        nc.scalar.activation(
                            azs[:, sl], pt[:, t * 129:t * 129 + 128],
                            AF.Copy, scale=w2[:, nb:nb + 1])
                    nc.vector.scalar_tensor_tensor(
                        rv_new[:, sl], hnat_sb[:, sl], w1[:, nb:nb + 1],
                        azs[:, sl], OP.mult, OP.add)
            if last:
                for half in range(2):
                    hsl = slice(half * 512, (half + 1) * 512)
                    nc.sync.dma_start(out[b, :, hsl], rv_new[:, hsl])
            else:
                st["rv"] = rv_new

        # phase-interleaved emission: engines run ahead independently, so
        # att(b1)'s ACT/DVE pipeline overlaps hop0(b0)'s PE burst.
        states = [{} for _ in range(BPC)]
        for b in range(BPC):
            phase_prologue(b, states[b])
        for b in range(BPC):
            phase_att(b, states[b])
        for k in range(3):
            for b in range(BPC):
                phase_hop(b, states[b], k)

        # Spare per-engine nops: relocated by _fixup_waits to carry sync
        # waits that walrus cannot fit on compute-instruction structs.
        nop_insts = []
        for eng in (nc.tensor, nc.vector, nc.scalar, nc.gpsimd, nc.sync):
            for _ in range(96):
                nop_insts.append(eng.nop(nofuse=True).ins)

    _fixup_waits(nc, nop_insts)
    return nc


_FIXUP_SKIP = {"InstNoOp"}


def _fixup_waits(nc, nop_insts):
    """walrus (enable-ldw-opt=false) rejects compute instructions with more
    than one sync wait (single wait slot in the S3 structs).  Hoist
    all-but-one wait of each such instruction onto spare same-engine nop
    instructions inserted immediately before it in program order."""
    nop_set = set(id(x) for x in nop_insts)
    free_nops = {}
    for x in nop_insts:
        free_nops.setdefault(x.engine, []).append(x)
    f = nc.m.functions[0]
    for blk in f.blocks:
        insts = blk.instructions
        for i in range(len(insts) - 1, -1, -1):
            if id(insts[i]) in nop_set:
                insts.pop(i)
        i = 0
        while i < len(insts):
            inst = insts[i]
            if inst.__class__.__name__ not in _FIXUP_SKIP:
                si = inst.sync_info
                if si is not None and si.on_wait and len(si.on_wait) > 1:
                    waits = list(si.on_wait)
                    extra, keep = waits[:-1], waits[-1:]
                    inst.sync_info = mybir.SyncInfo(
                        on_wait=keep, on_update=list(si.on_update or []))
                    pool = free_nops.get(inst.engine)
                    for k, w in enumerate(extra):
                        if not pool:
                            raise RuntimeError(
                                f"out of spare nops for {inst.engine}")
                        nop = pool.pop()
                        nop.sync_info = mybir.SyncInfo(on_wait=[w], on_update=[])
                        insts.insert(i + k, nop)
                    i += len(extra)
            i += 1


_NC_CACHE = None


def _get_nc():
    global _NC_CACHE
    if _NC_CACHE is None:
        _NC_CACHE = build_nc()
    return _NC_CACHE


def _prep_in_maps(inputs):
    x = np.ascontiguousarray(np.asarray(inputs["x"], dtype=np.float32))
    adj = np.ascontiguousarray(np.asarray(inputs["adj"], dtype=np.float32))
    W_w = np.asarray(inputs["W_w"], dtype=np.float32)
    W_b = np.asarray(inputs["W_b"], dtype=np.float32)
    A = np.asarray(inputs["A"], dtype=np.float32)
    gate_w = np.asarray(inputs["gate_w"], dtype=np.float32)
    gate_b = np.asarray(inputs["gate_b"], dtype=np.float32)

    S = (A + A.T).astype(np.float64)
    Wd, bd = W_w.astype(np.float64), W_b.astype(np.float64)
    M = (Wd.T @ S @ Wd)
    v = Wd.T @ S @ bd
    c0 = float(bd @ S @ bd)

    cblob = np.zeros((128, C_COLS), dtype=np.float32)
    cblob[:, C_ID:C_ID + 128] = np.eye(128, dtype=np.float32)
    cblob[:, C_WW:C_WW + 128] = W_w.T
    cblob[:, C_WB] = W_b
    cblob[:, C_A:C_A + 128] = M.astype(np.float32)
    cblob[:, C_GW:C_GW + 2] = gate_w.reshape(2, D).T
    cblob[:, C_NGB] = -float(gate_b.reshape(()))
    cblob[:, C_V] = v.astype(np.float32)

    in_maps = []
    for c in range(NCORES):
        sl = slice(c * BPC, (c + 1) * BPC)
        adj_c = adj[sl]
        # adjP[b, p, jb*N + i] = adj[i, jb*128+p], as bf16 bits (0/1 exact)
        adjT_c = adj_c.transpose(0, 2, 1)                          # [BPC, j, i]
        adjP_c = np.ascontiguousarray(
            adjT_c.reshape(BPC, NB, 128, N).transpose(0, 2, 1, 3)
            .reshape(BPC, 128, NB * N))
        adjP_bits = (adjP_c != 0).astype(np.uint8)
        xT_c = x[sl].transpose(0, 2, 1)                            # [BPC, D, N]
        ndeg = (N - adj_c.sum(axis=1)).astype(np.float32)          # [BPC, N]
        ndegT = ndeg.reshape(BPC, NB, 128).transpose(0, 2, 1)      # [BPC, 128, NB]
        q = (x[sl].astype(np.float64) @ v + c0).astype(np.float32)  # [BPC, N]
        qT = q.reshape(BPC, NB, 128).transpose(0, 2, 1)             # [BPC, 128, NB]
        xTn_c = np.ascontiguousarray(
            np.concatenate([xT_c, ndegT, qT], axis=2))             # [BPC, D, N+2NB]
        in_maps.append({
            "cblob": cblob, "xTn": xTn_c, "adjP": adjP_bits,
        })
    return in_maps


def _run(inputs, trace=False, **kwargs):
    nc = _get_nc()
    in_maps = _prep_in_maps(inputs)
    res = run_bass_kernel_spmd(nc, in_maps, core_ids=list(range(NCORES)),
                               trace=trace, **kwargs)
    # out[b, p, nb*128+f] holds rv[node=nb*128+p, f]: un-permute on host
    outs = []
    for c in range(NCORES):
        o = res.results[c]["out"].reshape(BPC, 128, NB, D)
        outs.append(np.ascontiguousarray(o.transpose(0, 2, 1, 3))
                    .reshape(BPC, N, D))
    out = np.concatenate(outs, axis=0)
    return out.astype(np.float32), res


def kernel(**inputs) -> np.ndarray:
    out, _ = _run(inputs, trace=False)
    return out

